# revision 62
# baseline (speedup 1.0000x reference)
"""Trainium2 Bass kernel for nn_Net_60413009985719.

Reference semantics: x[L] -> 5 stacked single-step LSTM cells (seq_len=1,
zero initial (h, c)) applied independently to every "batch" row, then the
head reads ONLY h[-1:].  Because h_prev = c_prev = 0, rows never interact:
the output depends solely on the scalar x[L-1].  The chosen sharding is the
degenerate limit of the data-parallel hint -- the shard owning the last row
is the only one with live work, so the kernel ships just that scalar (plus
the tiny weights) to the device and runs the 5-cell + MLP-head chain there.

Optimizations (27.1us session-measured baseline -> ~19.3us):
- fp16 weights/activations on the matmul datapath (max rel err 1.4e-3 vs
  the 2e-2 tolerance; fp16's round-to-nearest beats bf16, which fails the
  gate at 2.3e-2).  Single-pass LDWEIGHTS/MATMUL at 1 cycle/row, no
  fp32r even-moving-dim or start-partition-0 constraints.
- Weight pre-scaling: h_k = sig(o_k) * tanh(c_k), so
  W @ [h; 1] == (W scaled per-contraction-row by sig(o)) @ [tanh(c); 1].
  For cells 2..4 and the fc matmul, a DVE scalar_tensor_tensor scales the
  weight columns in place by sig(o) right after the Sigmoid lands --
  OFF the critical path -- and ACT's Tanh(c) writes straight into the rhs
  template column, removing the serial ACT->DVE->PE h-product round trip.
  Cell 1 keeps the explicit h product (its weights arrive by DMA too
  late to pre-scale without stalling).
- Row-major head: after z = relu(fc' @ [tc4; 1]), ONE matmul with the
  [z; 1] column as the stationary operand against [c1row | unit | mean |
  ls] yields a [1, 19] row on PSUM partition 0 (u_pre + unit + mean +
  ls).  A single DVE scalar_tensor_tensor(op0=max, op1=mult, accum_out)
  then computes v = sum(c2 . relu(u)) -- relu, product and reduction in
  one op -- and the result row is a single-partition 12-byte DMA.
- Input DMAs issued ahead of the Block (right after SP's preamble);
  4 chunks ordered by first use so only the 20-byte-per-partition first
  chunk gates the chain start.
- Layer-0 gates are affine in the single live scalar x, so the host ships
  g0 = Wih0*x + bih0 + bhh0; the ACT chain starts when the first (tiny)
  DMA chunk lands.
- The f-gate is dead (f * c_prev == 0) and is never computed.
- ACT table load (sigmoid set) triggered at t=0 by a dependency-free
  warm-up op; tanh(g) scratch lives in PSUM (cheaper ACT access).
- Raw Bass; csem orders the strictly-alternating ACT/PE chain, vsem
  orders the concurrent DVE weight-scaling ops.

The same tiny program runs SPMD on all 8 cores (replicated); core 0's
output is returned.
"""

import numpy as np

import concourse.bass as bass
from concourse import mybir
from concourse.bass_utils import run_bass_kernel_spmd

F32 = mybir.dt.float32
F16 = mybir.dt.float16
AF = mybir.ActivationFunctionType
ALU = mybir.AluOpType

H = 64          # hidden size
K = H + 1       # contraction dim: hidden + bias row
L = 500_000     # full input length

# column map inside the packed tensor wp [65, _WP_COLS]
_COL_H = 0                  # tanh(c) / h rhs cols for cells 0..4 (1.0 at row 64)
_COL_V = 5                  # z rhs col (1.0 at row 32 for the K=33 head matmuls)
_COL_G0 = 7                 # layer-0 gates [i0, o0, g0] (3 cols, host-computed)
_COL_L1 = 10                # cells 1..4 weights (4 x 192 cols: gate blocks i, o, g)
_COL_FC = _COL_L1 + 4 * 192     # 778
_COL_C1R = _COL_FC + 32         # 810  c1 row-major rhs [33, 17] (col 16 = bias unit)
_COL_ML = _COL_C1R + 17         # 827  [mean | ls] lhsT [33, 2]
_COL_C2R = _COL_ML + 2          # 829  c2 row (17 cols on partition 0 only)
_WP_COLS = 848

_CHUNK1 = _COL_L1               # cols 0:10    templates + g0 (critical)
_CHUNK2 = _COL_L1 + 192         # cols 10:202  W_1
_CHUNK3 = _COL_L1 + 3 * 192     # cols 202:586 W_2, W_3
_NW = _COL_C2R + 17             # cols 586:846 W_4, fc, head blocks

_CACHE = {}


def _pack_weights(inputs):
    """Pack all lhsT blocks: rows 0:64 = W.T, row 64 = bias.  Layer-0 is
    shipped as precomputed gate pre-activations (affine in the one live
    scalar x[L-1]).  Returned as fp16."""
    wp = np.zeros((K, _WP_COLS), np.float32)

    x = np.float32(np.asarray(inputs["x"])[L - 1])
    g0 = (
        np.asarray(inputs["Wih0"], np.float32)[:, 0] * x
        + np.asarray(inputs["bih0"], np.float32)
        + np.asarray(inputs["bhh0"], np.float32)
    )
    # the device only needs cell 0's i and g pre-activations; sig(o0) is
    # host-known and folded straight into W_1's contraction rows below
    wp[0:H, _COL_G0] = g0[0:64]         # i0
    wp[0:H, _COL_G0 + 1] = g0[128:192]  # g0
    sig_o0 = 1.0 / (1.0 + np.exp(-g0[192:256]))

    def put(col, w_t, bias, row0=0):
        wp[row0 : row0 + w_t.shape[0], col : col + w_t.shape[1]] = w_t
        wp[H, col : col + w_t.shape[1]] = bias

    for l in range(4):
        w = np.asarray(inputs["Wih"][l], np.float32)          # [256, 64]
        b = np.asarray(inputs["bih"][l], np.float32) + np.asarray(
            inputs["bhh"][l], np.float32
        )
        base = _COL_L1 + l * 192
        for gi, rows in enumerate((slice(0, 64), slice(192, 256), slice(128, 192))):
            put(base + gi * 64, w[rows].T, b[rows])
    # cell 1 consumes [tanh(c0); 1] directly: W_1 rows pre-scaled by sig(o0)
    wp[0:H, _COL_L1 : _COL_L1 + 192] *= sig_o0[:, None]

    put(_COL_FC, np.asarray(inputs["fc_w"], np.float32).T,
        np.asarray(inputs["fc_b"], np.float32))
    # c1 row-major: rhs[k, j] = c1_w[j, k], bias at row 32; col 16 is the
    # bias-unit column ([0]*32 + [1.0] at row 32) so relu(u)[16] == 1.0
    c1_w = np.asarray(inputs["c1_w"], np.float32)          # [16, 32]
    wp[0:32, _COL_C1R : _COL_C1R + 16] = c1_w.T
    wp[32, _COL_C1R : _COL_C1R + 16] = np.asarray(inputs["c1_b"], np.float32)
    wp[32, _COL_C1R + 16] = 1.0
    # [mean | ls] lhsT over [z; 1]: rows 0:32 = w.T, row 32 = bias
    wp[0:32, _COL_ML] = np.asarray(inputs["mean_w"], np.float32)[0]
    wp[32, _COL_ML] = np.asarray(inputs["mean_b"], np.float32)[0]
    wp[0:32, _COL_ML + 1] = np.asarray(inputs["ls_w"], np.float32)[0]
    wp[32, _COL_ML + 1] = np.asarray(inputs["ls_b"], np.float32)[0]
    # c2 as a row on partition 0 (pairs with the row-major head outputs);
    # col 16 = c2_b, dotted against relu(u)[16] == 1.0
    wp[0, _COL_C2R : _COL_C2R + 16] = np.asarray(inputs["c2_w"], np.float32)[0]
    wp[0, _COL_C2R + 16] = np.asarray(inputs["c2_b"], np.float32)[0]

    # rhs templates: zeros with the bias-partner 1.0 in row 64
    wp[H, _COL_H : _COL_V] = 1.0       # tc cols
    wp[32, _COL_V] = 1.0               # z col pairs with K=33 head matmuls

    return wp.astype(np.float16)


def _build_program():
    nc = bass.Bass()
    wp_d = nc.declare_dram_parameter("wp", [K, _WP_COLS], F16, isOutput=False)
    out_d = nc.declare_dram_parameter("out", [3, 1], F32, isOutput=True)

    with (
        nc.sbuf_tensor("WALL", [K, _NW], F16) as WALL,
        nc.sbuf_tensor("A", [H, 8], F32) as A,   # 2 rotating sets of sig_i, sig_o, t_g, t_c
        nc.sbuf_tensor("warm", [1, 2], F32) as warm,
        nc.sbuf_tensor("res", [1, 4], F32) as res,   # [mean, ls, v] as one row
        nc.sbuf_tensor("ROW", [1, 34], F16) as ROW,  # relu(u) row + ttreduce scratch
        nc.psum_tensor("PS", [H, 40], F32) as PS,  # 4x3 gate cols + fc, head
        nc.psum_tensor("PG", [H, 2], F32) as PG,   # tanh(g) scratch (psum is
                                                   # cheaper for ACT access)
        nc.semaphore("dsem") as dsem,
        nc.semaphore("csem") as csem,
        nc.semaphore("vsem") as vsem,
    ):
        w = [WALL[:, _COL_L1 + l * 192 : _COL_L1 + (l + 1) * 192] for l in range(4)]

        def mm(out, lhsT, rhs):
            return nc.tensor.matmul(out, lhsT, rhs, start=True, stop=True)

        # input DMAs issued ahead of the Block (right after SP's preamble)
        nc.sync.dma_start(out=WALL[:, :_CHUNK1],
                          in_=wp_d[:, :_CHUNK1]).then_inc(dsem, 16)
        nc.sync.dma_start(out=WALL[:, _CHUNK1:_CHUNK2],
                          in_=wp_d[:, _CHUNK1:_CHUNK2]).then_inc(dsem, 16)
        nc.sync.dma_start(out=WALL[:, _CHUNK2:_CHUNK3],
                          in_=wp_d[:, _CHUNK2:_CHUNK3]).then_inc(dsem, 16)
        nc.sync.dma_start(out=WALL[:, _CHUNK3:_NW],
                          in_=wp_d[:, _CHUNK3:_NW]).then_inc(dsem, 16)

        with nc.Block(no_gpsimd_drain=True) as block:
            @block.sync
            def _(sync):
                sync.wait_ge(vsem, 9)                # res fully written (DVE)
                # 12-byte result: sequencer reg_load/store beats a DMA here --
                # no descriptor generation and no completion drain gating the
                # teardown barrier
                U32 = mybir.dt.uint32
                for j in range(3):
                    r = nc.sync.alloc_register(f"res_out_{j}")
                    nc.sync.reg_load(r, res[0:1, j : j + 1].bitcast(U32))
                    nc.sync.store(out_d[j : j + 1, 0:1].bitcast(U32), r)

            @block.tensor
            def _(pe):
                # cell 1: gates = (W_1 . sig_o0) @ [tanh(c0); 1]; sig(o0)
                # was folded into W_1 on the host
                pe.wait_ge(csem, 1)                  # t_c0 done (ACT)
                pe.wait_ge(dsem, 32)                 # W_1 weights
                rhs = WALL[:, _COL_H : _COL_H + 1]
                mm(PS[:, 0:1], w[0][:, 0:64], rhs)                          # i
                mm(PS[:, 1:2], w[0][:, 64:128], rhs).then_inc(csem, 1)      # o -> 2
                mm(PS[:, 2:3], w[0][:, 128:192], rhs).then_inc(csem, 1)     # g -> 3
                # cells 2..4: gates = (W_l . sig_o) @ [tanh(c); 1]
                for l in range(1, 4):
                    pe.wait_ge(csem, 1 + 4 * l)      # t_c_l written to rhs col
                    pe.wait_ge(vsem, 2 * l - 1)      # W'_io scaled
                    rhs = WALL[:, _COL_H + l : _COL_H + l + 1]
                    ps = PS[:, 3 * l : 3 * l + 3]
                    mm(ps[:, 0:1], w[l][:, 0:64], rhs)                      # i
                    mm(ps[:, 1:2], w[l][:, 64:128], rhs).then_inc(csem, 1)  # o
                    pe.wait_ge(vsem, 2 * l)          # W'_g scaled
                    mm(ps[:, 2:3], w[l][:, 128:192], rhs).then_inc(csem, 1) # g
                pe.wait_ge(csem, 17)                 # t_c4 written
                pe.wait_ge(vsem, 7)                  # fc' scaled
                mm(PS[0:32, 15:16], WALL[:, _COL_FC : _COL_FC + 32],
                   WALL[:, _COL_H + 4 : _COL_H + 5]).then_inc(csem, 1)      # 18 fc
                pe.wait_ge(csem, 19)                 # z ready
                # one row-major matmul: [z; 1] as the stationary column
                # against [c1row | unit | mean | ls] -> [1, 19] on PSUM
                # partition 0: cols 0:17 = u_pre row (+unit), 17 = mean, 18 = ls
                mm(PS[0:1, 19:38], WALL[0:33, _COL_V : _COL_V + 1],
                   WALL[0:33, _COL_C1R : _COL_C1R + 19]).then_inc(csem, 1)  # 20

            @block.scalar
            def _(act):
                # dependency-free warm-up: triggers the sigmoid/tanh table
                # load at t=0; scale=0.0 zeroes the (uninitialized) input
                nc.scalar.activation(warm[0:1, 1:2], warm[0:1, 0:1],
                                     AF.Sigmoid, scale=0.0)
                # cell 0: i0/g0 pre-activations host-computed into WALL cols
                act.wait_ge(dsem, 16)
                nc.scalar.activation(A[:, 0:1],
                                     WALL[0:H, _COL_G0 : _COL_G0 + 1],
                                     AF.Sigmoid)
                nc.scalar.activation(PG[:, 0:1],
                                     WALL[0:H, _COL_G0 + 1 : _COL_G0 + 2],
                                     AF.Tanh)
                # tanh(c0) straight into cell 1's rhs column
                nc.scalar.activation(WALL[0:H, _COL_H : _COL_H + 1],
                                     PG[:, 0:1], AF.Tanh,
                                     scale=A[:, 0:1]).then_inc(csem, 1)  # t_c0 -> 1
                for l in range(4):
                    a = A[:, 4 * ((l + 1) % 2) : 4 * ((l + 1) % 2) + 4]
                    ps = PS[:, 3 * l : 3 * l + 3]
                    act.wait_ge(csem, 2 + 4 * l)     # i, o landed; overlaps g mm
                    nc.scalar.activation(a[:, 0:2], ps[:, 0:2],
                                         AF.Sigmoid).then_inc(csem, 1)   # 4+4l
                    act.wait_ge(csem, 3 + 4 * l)     # g landed
                    nc.scalar.activation(PG[:, 0:1], ps[:, 2:3], AF.Tanh)
                    # tanh(c) straight into the rhs template column
                    nc.scalar.activation(WALL[0:H, _COL_H + l + 1 : _COL_H + l + 2],
                                         PG[:, 0:1], AF.Tanh,
                                         scale=a[:, 0:1]).then_inc(csem, 1)  # 5+4l

            @block.vector
            def _(dve):
                # cells 2..4 + fc: scale weights in place by sig_o
                for l in range(1, 4):
                    a = A[:, 4 * (l % 2) : 4 * (l % 2) + 4]
                    dve.wait_ge(csem, 4 * l)         # S_io of cell l done
                    if l == 1:
                        dve.wait_ge(dsem, 48)        # W_2 (+W_3) chunk landed
                    elif l == 3:
                        dve.wait_ge(dsem, 64)        # W_4 chunk landed
                    nc.vector.scalar_tensor_tensor(
                        w[l][0:H, 0:128], w[l][0:H, 0:128],
                        a[:, 1:2], w[l][0:H, 0:128],
                        op0=ALU.mult, op1=ALU.bypass).then_inc(vsem, 1)  # 2l-1
                    nc.vector.scalar_tensor_tensor(
                        w[l][0:H, 128:192], w[l][0:H, 128:192],
                        a[:, 1:2], w[l][0:H, 128:192],
                        op0=ALU.mult, op1=ALU.bypass).then_inc(vsem, 1)  # 2l
                dve.wait_ge(csem, 16)                # S_io of cell 4 done
                nc.vector.scalar_tensor_tensor(
                    WALL[0:H, _COL_FC : _COL_FC + 32],
                    WALL[0:H, _COL_FC : _COL_FC + 32],
                    A[:, 1:2],
                    WALL[0:H, _COL_FC : _COL_FC + 32],
                    op0=ALU.mult, op1=ALU.bypass).then_inc(vsem, 1)      # 7 fc'
                dve.wait_ge(csem, 18)
                nc.vector.tensor_relu(WALL[0:32, _COL_V : _COL_V + 1],
                                      PS[0:32, 15:16]).then_inc(csem, 1)     # 19 z
                dve.wait_ge(csem, 20)
                # v = sum(relu(u_pre) . c2row) in one op: out = (u max 0) * c2,
                # accum_out = sum(out) -> res[0, 2]
                nc.vector.scalar_tensor_tensor(
                    ROW[0:1, 0:17], PS[0:1, 19:36], 0.0,
                    WALL[0:1, _COL_C2R : _COL_C2R + 17],
                    op0=ALU.max, op1=ALU.mult,
                    accum_out=res[0:1, 2:3]).then_inc(vsem, 1)               # 8 v
                nc.vector.tensor_copy(res[0:1, 0:2],
                                      PS[0:1, 36:38]).then_inc(vsem, 1)      # 9 mean,ls

    return nc


def kernel(**inputs):
    if "nc" not in _CACHE:
        _CACHE["nc"] = _build_program()
    nc = _CACHE["nc"]

    wp = _pack_weights(inputs)

    in_maps = [{"wp": wp} for _ in range(8)]
    res = run_bass_kernel_spmd(nc, in_maps, list(range(8)))
    out = np.asarray(res.results[0]["out"], np.float32)  # [3, 1]
    return (out[0:1, :], out[1:2, :], out[2:3, :])


# revision 63
# speedup vs baseline: 1.1996x; 1.1996x over previous
"""Trainium2 Bass kernel for nn_Net_60413009985719.

Reference semantics: x[L] -> 5 stacked single-step LSTM cells (seq_len=1,
zero initial (h, c)) applied independently to every "batch" row, then the
head reads ONLY h[-1:].  Because h_prev = c_prev = 0, rows never interact:
the output depends solely on the scalar x[L-1].  The chosen sharding is the
degenerate limit of the data-parallel hint -- the shard owning the last row
is the only one with live work, so the kernel ships just that scalar (plus
the tiny weights) to the device and runs the 5-cell + MLP-head chain there.

Optimizations (27.1us session-measured baseline -> ~19.3us):
- fp16 weights/activations on the matmul datapath (max rel err 1.4e-3 vs
  the 2e-2 tolerance; fp16's round-to-nearest beats bf16, which fails the
  gate at 2.3e-2).  Single-pass LDWEIGHTS/MATMUL at 1 cycle/row, no
  fp32r even-moving-dim or start-partition-0 constraints.
- Weight pre-scaling: h_k = sig(o_k) * tanh(c_k), so
  W @ [h; 1] == (W scaled per-contraction-row by sig(o)) @ [tanh(c); 1].
  For cells 2..4 and the fc matmul, a DVE scalar_tensor_tensor scales the
  weight columns in place by sig(o) right after the Sigmoid lands --
  OFF the critical path -- and ACT's Tanh(c) writes straight into the rhs
  template column, removing the serial ACT->DVE->PE h-product round trip.
  Cell 1 keeps the explicit h product (its weights arrive by DMA too
  late to pre-scale without stalling).
- Row-major head: after z = relu(fc' @ [tc4; 1]), ONE matmul with the
  [z; 1] column as the stationary operand against [c1row | unit | mean |
  ls] yields a [1, 19] row on PSUM partition 0 (u_pre + unit + mean +
  ls).  A single DVE scalar_tensor_tensor(op0=max, op1=mult, accum_out)
  then computes v = sum(c2 . relu(u)) -- relu, product and reduction in
  one op -- and the result row is a single-partition 12-byte DMA.
- Input DMAs issued ahead of the Block (right after SP's preamble);
  4 chunks ordered by first use so only the 20-byte-per-partition first
  chunk gates the chain start.
- Layer-0 gates are affine in the single live scalar x, so the host ships
  g0 = Wih0*x + bih0 + bhh0; the ACT chain starts when the first (tiny)
  DMA chunk lands.
- The f-gate is dead (f * c_prev == 0) and is never computed.
- ACT table load (sigmoid set) triggered at t=0 by a dependency-free
  warm-up op; tanh(g) scratch lives in PSUM (cheaper ACT access).
- Raw Bass; csem orders the strictly-alternating ACT/PE chain, vsem
  orders the concurrent DVE weight-scaling ops.

The same tiny program runs SPMD on all 8 cores (replicated); core 0's
output is returned.
"""

import numpy as np

import concourse.bass as bass
from concourse import mybir
from concourse.bass_utils import run_bass_kernel_spmd

F32 = mybir.dt.float32
F16 = mybir.dt.float16
AF = mybir.ActivationFunctionType
ALU = mybir.AluOpType

H = 64          # hidden size
K = H + 1       # contraction dim: hidden + bias row
L = 500_000     # full input length

# column map inside the packed tensor wp [65, _WP_COLS]
_COL_H = 0                  # tanh(c) / h rhs cols for cells 0..4 (1.0 at row 64)
_COL_V = 5                  # z rhs col (1.0 at row 32 for the K=33 head matmuls)
_COL_G0 = 7                 # layer-0 gates [i0, o0, g0] (3 cols, host-computed)
_COL_L1 = 10                # cells 1..4 weights (4 x 192 cols: gate blocks i, o, g)
_COL_FC = _COL_L1 + 4 * 192     # 778
_COL_C1R = _COL_FC + 32         # 810  c1 row-major rhs [33, 17] (col 16 = bias unit)
_COL_ML = _COL_C1R + 17         # 827  [mean | ls] lhsT [33, 2]
_COL_C2R = _COL_ML + 2          # 829  c2 row (17 cols on partition 0 only)
_WP_COLS = 848

_CHUNK1 = _COL_L1               # cols 0:10    templates + g0 (critical)
_CHUNK2 = _COL_L1 + 192         # cols 10:202  W_1
_CHUNK3 = _COL_L1 + 3 * 192     # cols 202:586 W_2, W_3
_NW = _COL_C2R + 17             # cols 586:846 W_4, fc, head blocks

_CACHE = {}


def _pack_weights(inputs):
    """Pack all lhsT blocks: rows 0:64 = W.T, row 64 = bias.  Layer-0 is
    shipped as precomputed gate pre-activations (affine in the one live
    scalar x[L-1]).  Returned as fp16."""
    wp = np.zeros((K, _WP_COLS), np.float32)

    x = np.float32(np.asarray(inputs["x"])[L - 1])
    g0 = (
        np.asarray(inputs["Wih0"], np.float32)[:, 0] * x
        + np.asarray(inputs["bih0"], np.float32)
        + np.asarray(inputs["bhh0"], np.float32)
    )
    # the device only needs cell 0's i and g pre-activations; sig(o0) is
    # host-known and folded straight into W_1's contraction rows below
    wp[0:H, _COL_G0] = g0[0:64]         # i0
    wp[0:H, _COL_G0 + 1] = g0[128:192]  # g0
    sig_o0 = 1.0 / (1.0 + np.exp(-g0[192:256]))

    def put(col, w_t, bias, row0=0):
        wp[row0 : row0 + w_t.shape[0], col : col + w_t.shape[1]] = w_t
        wp[H, col : col + w_t.shape[1]] = bias

    for l in range(4):
        w = np.asarray(inputs["Wih"][l], np.float32)          # [256, 64]
        b = np.asarray(inputs["bih"][l], np.float32) + np.asarray(
            inputs["bhh"][l], np.float32
        )
        base = _COL_L1 + l * 192
        for gi, rows in enumerate((slice(0, 64), slice(192, 256), slice(128, 192))):
            put(base + gi * 64, w[rows].T, b[rows])
    # cell 1 consumes [tanh(c0); 1] directly: W_1 rows pre-scaled by sig(o0)
    wp[0:H, _COL_L1 : _COL_L1 + 192] *= sig_o0[:, None]

    put(_COL_FC, np.asarray(inputs["fc_w"], np.float32).T,
        np.asarray(inputs["fc_b"], np.float32))
    # c1 row-major: rhs[k, j] = c1_w[j, k], bias at row 32; col 16 is the
    # bias-unit column ([0]*32 + [1.0] at row 32) so relu(u)[16] == 1.0
    c1_w = np.asarray(inputs["c1_w"], np.float32)          # [16, 32]
    wp[0:32, _COL_C1R : _COL_C1R + 16] = c1_w.T
    wp[32, _COL_C1R : _COL_C1R + 16] = np.asarray(inputs["c1_b"], np.float32)
    wp[32, _COL_C1R + 16] = 1.0
    # [mean | ls] lhsT over [z; 1]: rows 0:32 = w.T, row 32 = bias
    wp[0:32, _COL_ML] = np.asarray(inputs["mean_w"], np.float32)[0]
    wp[32, _COL_ML] = np.asarray(inputs["mean_b"], np.float32)[0]
    wp[0:32, _COL_ML + 1] = np.asarray(inputs["ls_w"], np.float32)[0]
    wp[32, _COL_ML + 1] = np.asarray(inputs["ls_b"], np.float32)[0]
    # c2 as a row on partition 0 (pairs with the row-major head outputs);
    # col 16 = c2_b, dotted against relu(u)[16] == 1.0
    wp[0, _COL_C2R : _COL_C2R + 16] = np.asarray(inputs["c2_w"], np.float32)[0]
    wp[0, _COL_C2R + 16] = np.asarray(inputs["c2_b"], np.float32)[0]

    # rhs templates: zeros with the bias-partner 1.0 in row 64
    wp[H, _COL_H : _COL_V] = 1.0       # tc cols
    wp[32, _COL_V] = 1.0               # z col pairs with K=33 head matmuls

    return wp.astype(np.float16)


def _build_program():
    nc = bass.Bass()
    wp_d = nc.declare_dram_parameter("wp", [K, _WP_COLS], F16, isOutput=False)
    out_d = nc.declare_dram_parameter("out", [3, 1], F32, isOutput=True)

    with (
        nc.sbuf_tensor("WALL", [K, _NW], F16) as WALL,
        nc.sbuf_tensor("A", [H, 8], F32) as A,   # 2 rotating sets of sig_i, sig_o, t_g, t_c
        nc.sbuf_tensor("warm", [1, 2], F32) as warm,
        nc.sbuf_tensor("res", [1, 4], F32) as res,   # [mean, ls, v] as one row
        nc.sbuf_tensor("ROW", [1, 34], F16) as ROW,  # relu(u) row + ttreduce scratch
        nc.psum_tensor("PS", [H, 40], F32) as PS,  # 4x3 gate cols + fc, head
        nc.psum_tensor("PG", [H, 2], F32) as PG,   # tanh(g) scratch (psum is
                                                   # cheaper for ACT access)
        nc.semaphore("dsem") as dsem,
        nc.semaphore("csem") as csem,
        nc.semaphore("vsem") as vsem,
    ):
        w = [WALL[:, _COL_L1 + l * 192 : _COL_L1 + (l + 1) * 192] for l in range(4)]

        def mm(out, lhsT, rhs):
            return nc.tensor.matmul(out, lhsT, rhs, start=True, stop=True)

        # input DMAs issued ahead of the Block (right after SP's preamble)
        nc.sync.dma_start(out=WALL[:, :_CHUNK1],
                          in_=wp_d[:, :_CHUNK1]).then_inc(dsem, 16)
        nc.sync.dma_start(out=WALL[:, _CHUNK1:_CHUNK2],
                          in_=wp_d[:, _CHUNK1:_CHUNK2]).then_inc(dsem, 16)
        nc.sync.dma_start(out=WALL[:, _CHUNK2:_CHUNK3],
                          in_=wp_d[:, _CHUNK2:_CHUNK3]).then_inc(dsem, 16)
        nc.sync.dma_start(out=WALL[:, _CHUNK3:_NW],
                          in_=wp_d[:, _CHUNK3:_NW]).then_inc(dsem, 16)

        with nc.Block(no_gpsimd_drain=True) as block:
            @block.sync
            def _(sync):
                sync.wait_ge(vsem, 9)                # res fully written (DVE)
                sync.dma_start(out=out_d[:, :], in_=res[0:1, 0:3],
                               single_packet=True).then_inc(dsem, 16)

            @block.tensor
            def _(pe):
                # cell 1: gates = (W_1 . sig_o0) @ [tanh(c0); 1]; sig(o0)
                # was folded into W_1 on the host
                pe.wait_ge(csem, 1)                  # t_c0 done (ACT)
                pe.wait_ge(dsem, 32)                 # W_1 weights
                rhs = WALL[:, _COL_H : _COL_H + 1]
                mm(PS[:, 0:1], w[0][:, 0:64], rhs)                          # i
                mm(PS[:, 1:2], w[0][:, 64:128], rhs).then_inc(csem, 1)      # o -> 2
                mm(PS[:, 2:3], w[0][:, 128:192], rhs).then_inc(csem, 1)     # g -> 3
                # cells 2..4: gates = (W_l . sig_o) @ [tanh(c); 1]
                for l in range(1, 4):
                    pe.wait_ge(csem, 1 + 4 * l)      # t_c_l written to rhs col
                    pe.wait_ge(vsem, 2 * l - 1)      # W'_io scaled
                    rhs = WALL[:, _COL_H + l : _COL_H + l + 1]
                    ps = PS[:, 3 * l : 3 * l + 3]
                    mm(ps[:, 0:1], w[l][:, 0:64], rhs)                      # i
                    mm(ps[:, 1:2], w[l][:, 64:128], rhs).then_inc(csem, 1)  # o
                    pe.wait_ge(vsem, 2 * l)          # W'_g scaled
                    mm(ps[:, 2:3], w[l][:, 128:192], rhs).then_inc(csem, 1) # g
                pe.wait_ge(csem, 17)                 # t_c4 written
                pe.wait_ge(vsem, 7)                  # fc' scaled
                mm(PS[0:32, 15:16], WALL[:, _COL_FC : _COL_FC + 32],
                   WALL[:, _COL_H + 4 : _COL_H + 5]).then_inc(csem, 1)      # 18 fc
                pe.wait_ge(csem, 19)                 # z ready
                # one row-major matmul: [z; 1] as the stationary column
                # against [c1row | unit | mean | ls] -> [1, 19] on PSUM
                # partition 0: cols 0:17 = u_pre row (+unit), 17 = mean, 18 = ls
                mm(PS[0:1, 19:38], WALL[0:33, _COL_V : _COL_V + 1],
                   WALL[0:33, _COL_C1R : _COL_C1R + 19]).then_inc(csem, 1)  # 20

            @block.scalar
            def _(act):
                # dependency-free warm-up: triggers the sigmoid/tanh table
                # load at t=0; scale=0.0 zeroes the (uninitialized) input
                nc.scalar.activation(warm[0:1, 1:2], warm[0:1, 0:1],
                                     AF.Sigmoid, scale=0.0)
                # cell 0: i0/g0 pre-activations host-computed into WALL cols
                act.wait_ge(dsem, 16)
                nc.scalar.activation(A[:, 0:1],
                                     WALL[0:H, _COL_G0 : _COL_G0 + 1],
                                     AF.Sigmoid)
                nc.scalar.activation(PG[:, 0:1],
                                     WALL[0:H, _COL_G0 + 1 : _COL_G0 + 2],
                                     AF.Tanh)
                # tanh(c0) straight into cell 1's rhs column
                nc.scalar.activation(WALL[0:H, _COL_H : _COL_H + 1],
                                     PG[:, 0:1], AF.Tanh,
                                     scale=A[:, 0:1]).then_inc(csem, 1)  # t_c0 -> 1
                for l in range(4):
                    a = A[:, 4 * ((l + 1) % 2) : 4 * ((l + 1) % 2) + 4]
                    ps = PS[:, 3 * l : 3 * l + 3]
                    act.wait_ge(csem, 2 + 4 * l)     # i, o landed; overlaps g mm
                    nc.scalar.activation(a[:, 0:2], ps[:, 0:2],
                                         AF.Sigmoid).then_inc(csem, 1)   # 4+4l
                    act.wait_ge(csem, 3 + 4 * l)     # g landed
                    nc.scalar.activation(PG[:, 0:1], ps[:, 2:3], AF.Tanh)
                    # tanh(c) straight into the rhs template column
                    nc.scalar.activation(WALL[0:H, _COL_H + l + 1 : _COL_H + l + 2],
                                         PG[:, 0:1], AF.Tanh,
                                         scale=a[:, 0:1]).then_inc(csem, 1)  # 5+4l

            @block.vector
            def _(dve):
                # cells 2..4 + fc: scale weights in place by sig_o
                for l in range(1, 4):
                    a = A[:, 4 * (l % 2) : 4 * (l % 2) + 4]
                    dve.wait_ge(csem, 4 * l)         # S_io of cell l done
                    if l == 1:
                        dve.wait_ge(dsem, 48)        # W_2 (+W_3) chunk landed
                    elif l == 3:
                        dve.wait_ge(dsem, 64)        # W_4 chunk landed
                    nc.vector.scalar_tensor_tensor(
                        w[l][0:H, 0:128], w[l][0:H, 0:128],
                        a[:, 1:2], w[l][0:H, 0:128],
                        op0=ALU.mult, op1=ALU.bypass).then_inc(vsem, 1)  # 2l-1
                    nc.vector.scalar_tensor_tensor(
                        w[l][0:H, 128:192], w[l][0:H, 128:192],
                        a[:, 1:2], w[l][0:H, 128:192],
                        op0=ALU.mult, op1=ALU.bypass).then_inc(vsem, 1)  # 2l
                dve.wait_ge(csem, 16)                # S_io of cell 4 done
                nc.vector.scalar_tensor_tensor(
                    WALL[0:H, _COL_FC : _COL_FC + 32],
                    WALL[0:H, _COL_FC : _COL_FC + 32],
                    A[:, 1:2],
                    WALL[0:H, _COL_FC : _COL_FC + 32],
                    op0=ALU.mult, op1=ALU.bypass).then_inc(vsem, 1)      # 7 fc'
                dve.wait_ge(csem, 18)
                nc.vector.tensor_relu(WALL[0:32, _COL_V : _COL_V + 1],
                                      PS[0:32, 15:16]).then_inc(csem, 1)     # 19 z
                dve.wait_ge(csem, 20)
                # v = sum(relu(u_pre) . c2row) in one op: out = (u max 0) * c2,
                # accum_out = sum(out) -> res[0, 2]
                nc.vector.scalar_tensor_tensor(
                    ROW[0:1, 0:17], PS[0:1, 19:36], 0.0,
                    WALL[0:1, _COL_C2R : _COL_C2R + 17],
                    op0=ALU.max, op1=ALU.mult,
                    accum_out=res[0:1, 2:3]).then_inc(vsem, 1)               # 8 v
                nc.vector.tensor_copy(res[0:1, 0:2],
                                      PS[0:1, 36:38]).then_inc(vsem, 1)      # 9 mean,ls

    return nc


def kernel(**inputs):
    if "nc" not in _CACHE:
        _CACHE["nc"] = _build_program()
    nc = _CACHE["nc"]

    wp = _pack_weights(inputs)

    in_maps = [{"wp": wp} for _ in range(8)]
    res = run_bass_kernel_spmd(nc, in_maps, list(range(8)))
    out = np.asarray(res.results[0]["out"], np.float32)  # [3, 1]
    return (out[0:1, :], out[1:2, :], out[2:3, :])
